# revision 3
# baseline (speedup 1.0000x reference)
"""Trainium2 Bass kernel for nn_ChunkedQuantHead (v2).

Computation (see reference):
  xc   = x.reshape(B, 16, 256)
  acts = mean(|xc|, axis=(0, 2))           # global per-chunk stat
  top4 = top_k(acts, 4)                    # global chunk routing
  routed = einsum('bkc,koc->bo', xc[:, top4], expert_w[top4]) + expert_b
  w_eff  = quant_w if max(acts) > 0.5 else sign(quant_w)*mean|quant_w|
  out    = routed @ w_eff.T + quant_b

Strategy (8 cores, data-parallel over batch):
  - Row permutation: tile t / partition p holds batch row 16*p + t, so
    the final outputs of all 16 tiles live in ONE [128, 160] tile and
    the out-DMA is 640B-contiguous per partition (vs 40B scatter).
  - Each core streams its 2048x4096 f32 shard ONCE (SWDGE f32->bf16
    cast).  Per tile: DVE per-chunk |x| partial sums, PE transposes
    (4 groups of 8 blocks), 32 projection matmuls -> y[b, c, o], then
    2 PE transposes store yT[(c,o), b] per chunk-half.  The last tile
    is split into 4 column-quarter DMAs so its stats land ~1.1us after
    the last byte instead of 4.4us.
  - ONE AllGather of the [1,16] |x| sums (DRAM bounce).  Top-4 is
    computed rank-style (outer compare matrix + row reduce), no serial
    max-peeling.  The masked combine + quantized head + bias collapse
    into 33 small PE matmuls into one [128, 160] PSUM tile:
      out = sum_g yT_g.T @ (mask_g * (wbinS + cond*dwS)) + ones x bias
    where the [80, 10] rhs and [1, 160] bias row are built on-chip
    from host-precomputed stacks.
"""

import numpy as np

import concourse.bacc as bacc
import concourse.tile as tile
import concourse.mybir as mybir
from concourse.bass_utils import run_bass_kernel_spmd

F32 = mybir.dt.float32
BF16 = mybir.dt.bfloat16
AX = mybir.AxisListType
OP = mybir.AluOpType

N_CORES = 8
B, F = 16384, 4096
CHUNKS, CHUNK, OUT = 16, 256, 10
TOPK = 4
THRESH = 0.5
BC = B // N_CORES            # 2048 rows per core
P = 128
TILES = BC // P              # 16 tiles of 128 rows
SUM_THRESH = THRESH * B * CHUNK  # compare sum(|x|) against this
GW = 8 * OUT                 # 80: rows of the stacked head weights

_CACHE = {}


def _build():
    nc = bacc.Bacc(
        "TRN2",
        target_bir_lowering=False,
        debug=False,
        num_devices=N_CORES,
    )

    x_d = nc.dram_tensor("x", [BC, F], F32, kind="ExternalInput")
    idb_d = nc.dram_tensor("id_bf", [P, P], BF16, kind="ExternalInput")
    idf_d = nc.dram_tensor("id_f32", [P, P], F32, kind="ExternalInput")
    # expert weights pre-arranged host-side: w_sb[p, h*160 + c*10 + o]
    #   = expert_w[c, o, h*128 + p]   (bf16)
    w_d = nc.dram_tensor("w_sb", [P, 2 * CHUNKS * OUT], BF16, kind="ExternalInput")
    # head weight stacks: wbinS/dwS[c'*10+o, o'] = wbin/dw[o', o]
    wbinS_d = nc.dram_tensor("wbinS", [GW, OUT], F32, kind="ExternalInput")
    dwS_d = nc.dram_tensor("dwS", [GW, OUT], F32, kind="ExternalInput")
    # mask expansion: Eg[c, c'*10+o] = 1 iff c == c' + 8*g
    e0_d = nc.dram_tensor("E0", [CHUNKS, GW], F32, kind="ExternalInput")
    e1_d = nc.dram_tensor("E1", [CHUNKS, GW], F32, kind="ExternalInput")
    # tiled identity for the bias row: IdT10[o, t*10+o'] = (o == o')
    idt_d = nc.dram_tensor("IdT10", [OUT, TILES * OUT], F32, kind="ExternalInput")
    # bias columns: r0 = (eb @ wbin.T + qb).T, r1 = (eb @ dw.T).T
    r0_d = nc.dram_tensor("r0", [OUT, 1], F32, kind="ExternalInput")
    r1_d = nc.dram_tensor("r1", [OUT, 1], F32, kind="ExternalInput")
    out_d = nc.dram_tensor("out", [BC, OUT], F32, kind="ExternalOutput")

    NQ = 4                    # column quarters for the last tile
    FQ = F // NQ              # 1024
    CQ = CHUNKS // NQ         # 4 chunks per quarter

    with tile.TileContext(nc) as tc:
        with (
            tc.tile_pool(name="const", bufs=1) as constp,
            tc.tile_pool(name="persist", bufs=1) as perp,
            tc.tile_pool(name="xb", bufs=4) as xbp,
            tc.tile_pool(name="xt", bufs=2) as xtp,
            tc.tile_pool(name="ysb", bufs=2) as ysp,
            tc.tile_pool(name="tail", bufs=1) as tailp,
            tc.tile_pool(name="ps_misc", bufs=1, space="PSUM") as psm,
            tc.tile_pool(name="dram", bufs=1, space="DRAM") as dramp,
        ):
            # ---- constants ----
            id_bf = constp.tile([P, P], BF16)
            nc.sync.dma_start(id_bf[:, :], idb_d.ap())
            id_f32 = constp.tile([P, P], F32)
            nc.sync.dma_start(id_f32[:, :], idf_d.ap())
            w_sb = constp.tile([P, 2 * CHUNKS * OUT], BF16)
            nc.sync.dma_start(w_sb[:, :], w_d.ap())
            wbinS = constp.tile([GW, OUT], F32)
            nc.sync.dma_start(wbinS[:, :], wbinS_d.ap())
            dwS = constp.tile([GW, OUT], F32)
            nc.sync.dma_start(dwS[:, :], dwS_d.ap())
            E0 = constp.tile([CHUNKS, GW], F32)
            nc.sync.dma_start(E0[:, :], e0_d.ap())
            E1 = constp.tile([CHUNKS, GW], F32)
            nc.sync.dma_start(E1[:, :], e1_d.ap())
            IdT10 = constp.tile([OUT, TILES * OUT], F32)
            nc.sync.dma_start(IdT10[:, :], idt_d.ap())
            r0c = constp.tile([OUT, 1], F32)
            nc.sync.dma_start(r0c[:, :], r0_d.ap())
            r1c = constp.tile([OUT, 1], F32)
            nc.sync.dma_start(r1c[:, :], r1_d.ap())
            ones_col = constp.tile([P, 1], F32)
            nc.vector.memset(ones_col[:, :], 1.0)
            ones_row = constp.tile([1, P], F32)
            nc.vector.memset(ones_row[:, :], 1.0)

            # persistent accumulators
            yT0 = perp.tile([GW, TILES * P], F32)   # chunks 0..7
            yT1 = perp.tile([GW, TILES * P], F32)   # chunks 8..15
            red_all = perp.tile([P, TILES * CHUNKS], F32)

            # DRAM bounce buffers for the AllGather
            cc_in = dramp.tile([1, CHUNKS], F32)
            cc_out = dramp.tile([N_CORES, CHUNKS], F32)

            # x rows viewed as [p, j, f] with batch row = 16*p + j
            x3 = x_d.ap().rearrange("(p j) f -> p j f", j=TILES)

            # ---- main pass over x: stats + all-chunk projection ----
            with (
                tc.tile_pool(name="ps_tr", bufs=2, space="PSUM") as pstr,
                tc.tile_pool(name="ps_y", bufs=2, space="PSUM") as psy,
                tc.tile_pool(name="ps_yt", bufs=2, space="PSUM") as psyt,
            ):
                def project_tile(t, xt):
                    """32 matmuls y[b, c, o] + 2 transposes -> yT slices."""
                    psy_t = psy.tile([P, CHUNKS * OUT], F32, tag="psy")
                    for c in range(CHUNKS):
                        for h in range(2):
                            kh = 2 * c + h
                            nc.tensor.matmul(
                                psy_t[:, c * OUT:(c + 1) * OUT],
                                lhsT=xt[:, kh * P:(kh + 1) * P],
                                rhs=w_sb[:, h * CHUNKS * OUT + c * OUT:
                                         h * CHUNKS * OUT + c * OUT + OUT],
                                start=(c == 0 and h == 0),
                                stop=(c == CHUNKS - 1 and h == 1),
                            )
                    y_sb = ysp.tile([P, CHUNKS * OUT], F32, tag="ysb")
                    nc.scalar.copy(y_sb[:, :], psy_t[:, :])
                    ps_t = psyt.tile([GW, 2 * P], F32, tag="psyt")
                    nc.tensor.transpose(
                        ps_t[:, 0:P], y_sb[:, 0:GW], id_f32[:, :]
                    )
                    nc.tensor.transpose(
                        ps_t[:, P:2 * P], y_sb[:, GW:2 * GW], id_f32[:, :]
                    )
                    nc.scalar.copy(yT0[:, t * P:(t + 1) * P], ps_t[:, 0:P])
                    nc.scalar.copy(yT1[:, t * P:(t + 1) * P], ps_t[:, P:2 * P])

                for t in range(TILES - 1):
                    xb = xbp.tile([P, F], BF16, tag="xb")
                    # SWDGE DMA with f32 -> bf16 cast in the datapath
                    nc.gpsimd.dma_start(xb[:, :], x3[:, t:t + 1, :])

                    # per-chunk sum of |x| for this tile (fused abs+reduce)
                    nc.vector.tensor_reduce(
                        red_all[:, t * CHUNKS:(t + 1) * CHUNKS],
                        xb[:, :].rearrange("p (c f) -> p c f", f=CHUNK),
                        axis=AX.X,
                        op=OP.add,
                        apply_absolute_value=True,
                    )

                    # transpose 32 blocks in 4 groups of 8: x[b,f] -> xT[f,b]
                    xt = xtp.tile([P, F], BF16, tag="xt")
                    for g in range(NQ):
                        ps = pstr.tile([P, 8 * P], BF16, tag="ps_tr")
                        for j in range(8):
                            k = 8 * g + j
                            nc.tensor.transpose(
                                ps[:, j * P:(j + 1) * P],
                                xb[:, k * P:(k + 1) * P],
                                id_bf[:, :],
                            )
                        nc.scalar.copy(
                            xt[:, g * 8 * P:(g + 1) * 8 * P], ps[:, :]
                        )
                    project_tile(t, xt)

                # ---- last tile: 4 column-quarter DMAs for a short stats
                # tail after the final byte lands ----
                t = TILES - 1
                xt = xtp.tile([P, F], BF16, tag="xt")
                for q in range(NQ):
                    xq = xbp.tile([P, FQ], BF16, tag=f"xq{q}")
                    nc.gpsimd.dma_start(
                        xq[:, :], x3[:, t:t + 1, q * FQ:(q + 1) * FQ]
                    )
                    nc.vector.tensor_reduce(
                        red_all[:, t * CHUNKS + q * CQ:
                                t * CHUNKS + (q + 1) * CQ],
                        xq[:, :].rearrange("p (c f) -> p c f", f=CHUNK),
                        axis=AX.X,
                        op=OP.add,
                        apply_absolute_value=True,
                    )
                    ps = pstr.tile([P, 8 * P], BF16, tag="ps_tr")
                    for j in range(8):
                        nc.tensor.transpose(
                            ps[:, j * P:(j + 1) * P],
                            xq[:, j * P:(j + 1) * P],
                            id_bf[:, :],
                        )
                    nc.scalar.copy(
                        xt[:, q * 8 * P:(q + 1) * 8 * P], ps[:, :]
                    )

                # global per-chunk sums for this core -> [1, 16] -> gather
                acts_p = tailp.tile([P, CHUNKS], F32, tag="acts_p")
                nc.vector.tensor_reduce(
                    acts_p[:, :],
                    red_all[:, :].rearrange("p (t c) -> p c t", c=CHUNKS),
                    axis=AX.X,
                    op=OP.add,
                )
                ps_a = psm.tile([1, CHUNKS], F32, tag="psmisc")
                nc.tensor.matmul(
                    ps_a[:, :], lhsT=ones_col[:, :], rhs=acts_p[:, :],
                    start=True, stop=True,
                )
                cc_sb = tailp.tile([1, CHUNKS], F32, tag="cc_sb")
                nc.vector.tensor_copy(cc_sb[:, :], ps_a[:, :])
                nc.sync.dma_start(cc_in[:, :], cc_sb[:, :])
                nc.gpsimd.collective_compute(
                    "AllGather",
                    OP.bypass,
                    replica_groups=[list(range(N_CORES))],
                    ins=[cc_in.opt()],
                    outs=[cc_out.opt()],
                )

                # last tile's projection overlaps the collective
                project_tile(t, xt)

            # ---- rank-based top-4 mask + head weights ----
            Sg = tailp.tile([N_CORES, CHUNKS], F32, tag="Sg")
            nc.sync.dma_start(Sg[:, :], cc_out[:, :])

            # S_row [1, 16] = total per-chunk sums
            ps_sr = psm.tile([1, CHUNKS], F32, tag="psmisc")
            nc.tensor.matmul(
                ps_sr[:, :], lhsT=ones_col[0:N_CORES, :], rhs=Sg[:, :],
                start=True, stop=True,
            )
            S_row = tailp.tile([1, CHUNKS], F32, tag="S_row")
            nc.vector.tensor_copy(S_row[:, :], ps_sr[:, :])
            # S_col [16, 1] via transpose + reduce
            ps_sgt = psm.tile([CHUNKS, N_CORES], F32, tag="ps2")
            nc.tensor.transpose(
                ps_sgt[:, :], Sg[:, :], id_f32[0:N_CORES, 0:N_CORES]
            )
            S_col = tailp.tile([CHUNKS, 1], F32, tag="S_col")
            nc.vector.tensor_reduce(
                S_col[:, :], ps_sgt[:, :], axis=AX.X, op=OP.add
            )
            # S_bcast[i, j] = S_j ; cmp[i, j] = (S_j > S_i); rank_i
            ps_bc = psm.tile([CHUNKS, CHUNKS], F32, tag="psmisc")
            nc.tensor.matmul(
                ps_bc[:, :], lhsT=ones_row[:, 0:CHUNKS], rhs=S_row[:, :],
                start=True, stop=True,
            )
            cmp = tailp.tile([CHUNKS, CHUNKS], F32, tag="cmp")
            nc.vector.tensor_scalar(
                cmp[:, :], ps_bc[:, :], S_col[:, :], None, op0=OP.is_gt
            )
            rank = tailp.tile([CHUNKS, 1], F32, tag="rank")
            nc.vector.tensor_reduce(rank[:, :], cmp[:, :], axis=AX.X, op=OP.add)
            mask_col = tailp.tile([CHUNKS, 1], F32, tag="mask_col")
            nc.vector.tensor_scalar(
                mask_col[:, :], rank[:, :], float(TOPK) - 0.5, None, op0=OP.is_lt
            )
            # cond = (max_c S_c > SUM_THRESH)
            m1 = tailp.tile([1, 1], F32, tag="m1")
            nc.vector.tensor_reduce(m1[:, :], S_row[:, :], axis=AX.X, op=OP.max)
            cond = tailp.tile([1, 1], F32, tag="cond")
            nc.vector.tensor_scalar(
                cond[:, :], m1[:, :], float(SUM_THRESH), None, op0=OP.is_gt
            )

            # mask80 per group, cond80, effective stacked head weights
            ps_m0 = psm.tile([GW, 1], F32, tag="psmisc")
            nc.tensor.matmul(
                ps_m0[:, :], lhsT=E0[:, :], rhs=mask_col[:, :],
                start=True, stop=True,
            )
            m80_0 = tailp.tile([GW, 1], F32, tag="m80_0")
            nc.vector.tensor_copy(m80_0[:, :], ps_m0[:, :])
            ps_m1 = psm.tile([GW, 1], F32, tag="ps2")
            nc.tensor.matmul(
                ps_m1[:, :], lhsT=E1[:, :], rhs=mask_col[:, :],
                start=True, stop=True,
            )
            m80_1 = tailp.tile([GW, 1], F32, tag="m80_1")
            nc.vector.tensor_copy(m80_1[:, :], ps_m1[:, :])
            ps_c80 = psm.tile([GW, 1], F32, tag="psmisc")
            nc.tensor.matmul(
                ps_c80[:, :], lhsT=ones_row[:, 0:GW], rhs=cond[:, :],
                start=True, stop=True,
            )
            c80 = tailp.tile([GW, 1], F32, tag="c80")
            nc.vector.tensor_copy(c80[:, :], ps_c80[:, :])

            weff = tailp.tile([GW, OUT], F32, tag="weff")
            nc.vector.tensor_scalar(
                weff[:, :], dwS[:, :], c80[:, :], None, op0=OP.mult
            )
            nc.vector.tensor_tensor(
                weff[:, :], weff[:, :], wbinS[:, :], op=OP.add
            )
            rhs0 = tailp.tile([GW, OUT], F32, tag="rhs0")
            nc.vector.tensor_scalar(
                rhs0[:, :], weff[:, :], m80_0[:, :], None, op0=OP.mult
            )
            rhs1 = tailp.tile([GW, OUT], F32, tag="rhs1")
            nc.vector.tensor_scalar(
                rhs1[:, :], weff[:, :], m80_1[:, :], None, op0=OP.mult
            )

            # bias row [1, 160]: (r0 + cond*r1) tiled over the 16 tiles
            ps_c10 = psm.tile([OUT, 1], F32, tag="ps2")
            nc.tensor.matmul(
                ps_c10[:, :], lhsT=ones_row[:, 0:OUT], rhs=cond[:, :],
                start=True, stop=True,
            )
            bias_col = tailp.tile([OUT, 1], F32, tag="bias_col")
            nc.vector.tensor_scalar(
                bias_col[:, :], r1c[:, :], ps_c10[:, :], None, op0=OP.mult
            )
            nc.vector.tensor_tensor(
                bias_col[:, :], bias_col[:, :], r0c[:, :], op=OP.add
            )
            ps_b160 = psm.tile([1, TILES * OUT], F32, tag="psmisc")
            nc.tensor.matmul(
                ps_b160[:, :], lhsT=bias_col[:, :], rhs=IdT10[:, :],
                start=True, stop=True,
            )
            b160 = tailp.tile([1, TILES * OUT], F32, tag="b160")
            nc.vector.tensor_copy(b160[:, :], ps_b160[:, :])

            # ---- combine + head + bias: 33 matmuls into one PSUM tile ----
            with tc.tile_pool(name="ps_out", bufs=1, space="PSUM") as pso:
                ps_out = pso.tile([P, TILES * OUT], F32, tag="ps_out")
                for t in range(TILES):
                    nc.tensor.matmul(
                        ps_out[:, t * OUT:(t + 1) * OUT],
                        lhsT=yT0[:, t * P:(t + 1) * P],
                        rhs=rhs0[:, :],
                        start=(t == 0),
                        stop=False,
                    )
                    nc.tensor.matmul(
                        ps_out[:, t * OUT:(t + 1) * OUT],
                        lhsT=yT1[:, t * P:(t + 1) * P],
                        rhs=rhs1[:, :],
                        start=False,
                        stop=False,
                    )
                nc.tensor.matmul(
                    ps_out[:, :], lhsT=ones_row[:, :], rhs=b160[:, :],
                    start=False, stop=True,
                )
                out_sb = tailp.tile([P, TILES * OUT], F32, tag="out_sb")
                nc.scalar.copy(out_sb[:, :], ps_out[:, :])
                # one DMA: per partition 16 consecutive rows = 640B contig
                nc.sync.dma_start(
                    out_d.ap().rearrange("(p j) o -> p j o", j=TILES),
                    out_sb[:, :].rearrange("p (j o) -> p j o", o=OUT),
                )

    nc.compile()
    return nc


def _get_nc():
    if "nc" not in _CACHE:
        _CACHE["nc"] = _build()
    return _CACHE["nc"]


def build_in_maps(x, expert_w, expert_b, quant_w, quant_b):
    import ml_dtypes

    x = np.ascontiguousarray(np.asarray(x, dtype=np.float32))
    expert_w = np.asarray(expert_w, dtype=np.float32)
    expert_b = np.asarray(expert_b, dtype=np.float32)
    quant_w = np.asarray(quant_w, dtype=np.float32)
    quant_b = np.asarray(quant_b, dtype=np.float32)

    # host-side weight prep (tiny tensors only)
    # w_sb[p, h*160 + c*10 + o] = expert_w[c, o, h*128 + p]
    wr = expert_w.reshape(CHUNKS, OUT, 2, P)            # c, o, h, p
    w_sb = np.ascontiguousarray(
        wr.transpose(3, 2, 0, 1).reshape(P, 2 * CHUNKS * OUT)
    ).astype(ml_dtypes.bfloat16)
    qmean = np.float32(np.mean(np.abs(quant_w)))
    wbin = (np.sign(quant_w) * qmean).astype(np.float32)
    dw = (quant_w - wbin).astype(np.float32)
    # stacks: wbinS[c'*10+o, o'] = wbin[o', o]  (i.e. wbin.T tiled 8x)
    wbinS = np.ascontiguousarray(np.tile(wbin.T, (8, 1))).astype(np.float32)
    dwS = np.ascontiguousarray(np.tile(dw.T, (8, 1))).astype(np.float32)
    # mask expansion consts
    E0 = np.zeros((CHUNKS, GW), np.float32)
    E1 = np.zeros((CHUNKS, GW), np.float32)
    for cp in range(8):
        E0[cp, cp * OUT:(cp + 1) * OUT] = 1.0
        E1[cp + 8, cp * OUT:(cp + 1) * OUT] = 1.0
    IdT10 = np.tile(np.eye(OUT, dtype=np.float32), (1, TILES))
    IdT10 = np.ascontiguousarray(IdT10)
    r0 = (expert_b @ wbin.T + quant_b).reshape(OUT, 1).astype(np.float32)
    r1 = (expert_b @ dw.T).reshape(OUT, 1).astype(np.float32)

    id_bf = np.eye(P, dtype=ml_dtypes.bfloat16)
    id_f32 = np.eye(P, dtype=np.float32)
    in_maps = []
    for i in range(N_CORES):
        in_maps.append({
            "x": np.ascontiguousarray(x[i * BC:(i + 1) * BC]),
            "w_sb": w_sb,
            "wbinS": wbinS,
            "dwS": dwS,
            "E0": E0,
            "E1": E1,
            "IdT10": IdT10,
            "r0": r0,
            "r1": r1,
            "id_bf": id_bf,
            "id_f32": id_f32,
        })
    return in_maps


def kernel(x, expert_w, expert_b, quant_w, quant_b):
    in_maps = build_in_maps(x, expert_w, expert_b, quant_w, quant_b)
    nc = _get_nc()
    res = run_bass_kernel_spmd(nc, in_maps, core_ids=list(range(N_CORES)))
    out = np.concatenate(
        [np.asarray(res.results[i]["out"]) for i in range(N_CORES)], axis=0
    )
    return out.astype(np.float32)


# revision 14
# speedup vs baseline: 1.2176x; 1.2176x over previous
"""Trainium2 Bass kernel for nn_ChunkedQuantHead (v2).

Computation (see reference):
  xc   = x.reshape(B, 16, 256)
  acts = mean(|xc|, axis=(0, 2))           # global per-chunk stat
  top4 = top_k(acts, 4)                    # global chunk routing
  routed = einsum('bkc,koc->bo', xc[:, top4], expert_w[top4]) + expert_b
  w_eff  = quant_w if max(acts) > 0.5 else sign(quant_w)*mean|quant_w|
  out    = routed @ w_eff.T + quant_b

Strategy (8 cores, data-parallel over batch):
  - Row permutation: tile t / partition p holds batch row 16*p + t, so
    the final outputs of all 16 tiles live in ONE [128, 160] tile and
    the out-DMA is 640B-contiguous per partition (vs 40B scatter).
  - Each core streams its 2048x4096 f32 shard ONCE (SWDGE f32->bf16
    cast).  Per tile: DVE per-chunk |x| partial sums, PE transposes
    (4 groups of 8 blocks), 32 projection matmuls -> y[b, c, o], then
    2 PE transposes store yT[(c,o), b] per chunk-half.  The last tile
    is split into 4 column-quarter DMAs so its stats land ~1.1us after
    the last byte instead of 4.4us.
  - ONE AllGather of the [1,16] |x| sums (DRAM bounce).  Top-4 is
    computed rank-style (outer compare matrix + row reduce), no serial
    max-peeling.  The masked combine + quantized head + bias collapse
    into 33 small PE matmuls into one [128, 160] PSUM tile:
      out = sum_g yT_g.T @ (mask_g * (wbinS + cond*dwS)) + ones x bias
    where the [80, 10] rhs and [1, 160] bias row are built on-chip
    from host-precomputed stacks.
"""

import numpy as np

import concourse.bacc as bacc
import concourse.tile as tile
import concourse.mybir as mybir
from concourse.bass_utils import run_bass_kernel_spmd

F32 = mybir.dt.float32
BF16 = mybir.dt.bfloat16
AX = mybir.AxisListType
OP = mybir.AluOpType

N_CORES = 8
B, F = 16384, 4096
CHUNKS, CHUNK, OUT = 16, 256, 10
TOPK = 4
THRESH = 0.5
BC = B // N_CORES            # 2048 rows per core
P = 128
TILES = BC // P              # 16 tiles of 128 rows
SUM_THRESH = THRESH * B * CHUNK  # compare sum(|x|) against this
GW = 8 * OUT                 # 80: rows of the stacked head weights

_CACHE = {}


def _build():
    nc = bacc.Bacc(
        "TRN2",
        target_bir_lowering=False,
        debug=False,
        num_devices=N_CORES,
    )

    x_d = nc.dram_tensor("x", [BC, F], F32, kind="ExternalInput")
    idb_d = nc.dram_tensor("id_bf", [P, P], BF16, kind="ExternalInput")
    idf_d = nc.dram_tensor("id_f32", [P, P], F32, kind="ExternalInput")
    # expert weights pre-arranged host-side: w_sb[p, h*160 + c*10 + o]
    #   = expert_w[c, o, h*128 + p]   (bf16)
    w_d = nc.dram_tensor("w_sb", [P, 2 * CHUNKS * OUT], BF16, kind="ExternalInput")
    # head weight stacks: wbinS/dwS[c'*10+o, o'] = wbin/dw[o', o]
    wbinS_d = nc.dram_tensor("wbinS", [GW, OUT], F32, kind="ExternalInput")
    dwS_d = nc.dram_tensor("dwS", [GW, OUT], F32, kind="ExternalInput")
    # mask expansion: Eg[c, c'*10+o] = 1 iff c == c' + 8*g
    e0_d = nc.dram_tensor("E0", [CHUNKS, GW], F32, kind="ExternalInput")
    e1_d = nc.dram_tensor("E1", [CHUNKS, GW], F32, kind="ExternalInput")
    # tiled identity for the bias row: IdT10[o, t*10+o'] = (o == o')
    idt_d = nc.dram_tensor("IdT10", [OUT, TILES * OUT], F32, kind="ExternalInput")
    # bias columns: r0 = (eb @ wbin.T + qb).T, r1 = (eb @ dw.T).T
    r0_d = nc.dram_tensor("r0", [OUT, 1], F32, kind="ExternalInput")
    r1_d = nc.dram_tensor("r1", [OUT, 1], F32, kind="ExternalInput")
    out_d = nc.dram_tensor("out", [BC, OUT], F32, kind="ExternalOutput")

    NQ = 4                    # column quarters for the last tile
    FQ = F // NQ              # 1024
    CQ = CHUNKS // NQ         # 4 chunks per quarter

    with tile.TileContext(nc) as tc:
        with (
            tc.tile_pool(name="const", bufs=1) as constp,
            tc.tile_pool(name="persist", bufs=1) as perp,
            tc.tile_pool(name="xb", bufs=4) as xbp,
            tc.tile_pool(name="xt", bufs=2) as xtp,
            tc.tile_pool(name="ysb", bufs=2) as ysp,
            tc.tile_pool(name="tail", bufs=1) as tailp,
            tc.tile_pool(name="ps_misc", bufs=1, space="PSUM") as psm,
            tc.tile_pool(name="dram", bufs=1, space="DRAM") as dramp,
        ):
            # ---- constants ----
            id_bf = constp.tile([P, P], BF16)
            nc.sync.dma_start(id_bf[:, :], idb_d.ap())
            id_f32 = constp.tile([P, P], F32)
            nc.sync.dma_start(id_f32[:, :], idf_d.ap())
            w_sb = constp.tile([P, 2 * CHUNKS * OUT], BF16)
            nc.sync.dma_start(w_sb[:, :], w_d.ap())
            wbinS = constp.tile([GW, OUT], F32)
            nc.sync.dma_start(wbinS[:, :], wbinS_d.ap())
            dwS = constp.tile([GW, OUT], F32)
            nc.sync.dma_start(dwS[:, :], dwS_d.ap())
            E0 = constp.tile([CHUNKS, GW], F32)
            nc.sync.dma_start(E0[:, :], e0_d.ap())
            E1 = constp.tile([CHUNKS, GW], F32)
            nc.sync.dma_start(E1[:, :], e1_d.ap())
            IdT10 = constp.tile([OUT, TILES * OUT], F32)
            nc.sync.dma_start(IdT10[:, :], idt_d.ap())
            r0c = constp.tile([OUT, 1], F32)
            nc.sync.dma_start(r0c[:, :], r0_d.ap())
            r1c = constp.tile([OUT, 1], F32)
            nc.sync.dma_start(r1c[:, :], r1_d.ap())
            ones_col = constp.tile([P, 1], F32)
            nc.vector.memset(ones_col[:, :], 1.0)
            ones_row = constp.tile([1, P], F32)
            nc.vector.memset(ones_row[:, :], 1.0)
            ones_row_bf = constp.tile([1, P], BF16)
            nc.vector.memset(ones_row_bf[:, :], 1.0)

            # persistent accumulators (bf16: feeds the tail matmuls)
            yT0 = perp.tile([GW, TILES * P], BF16)   # chunks 0..7
            yT1 = perp.tile([GW, TILES * P], BF16)   # chunks 8..15
            red_all = perp.tile([P, TILES * CHUNKS], F32)
            acts_e = perp.tile([P, CHUNKS], F32)

            # DRAM bounce buffers for the AllGathers
            cc_w_in = dramp.tile([1, CHUNKS], F32)
            cc_w_out = dramp.tile([N_CORES, CHUNKS], F32)
            cc_in = dramp.tile([1, CHUNKS], F32)
            cc_out = dramp.tile([N_CORES, CHUNKS], F32)

            # warmup AllGather: pays the one-time CC stream init (~30us)
            # under the x stream so the real gather runs the fast path
            warm_sb = constp.tile([1, CHUNKS], F32)
            nc.vector.memset(warm_sb[:, :], 0.0)
            nc.sync.dma_start(cc_w_in[:, :], warm_sb[:, :])
            nc.gpsimd.collective_compute(
                "AllGather",
                OP.bypass,
                replica_groups=[list(range(N_CORES))],
                ins=[cc_w_in.opt()],
                outs=[cc_w_out.opt()],
            )

            # x rows viewed as [p, j, f] with batch row = 16*p + j
            x3 = x_d.ap().rearrange("(p j) f -> p j f", j=TILES)

            # ---- main pass over x: stats + all-chunk projection ----
            with (
                tc.tile_pool(name="ps_tr", bufs=2, space="PSUM") as pstr,
                tc.tile_pool(name="ps_y", bufs=2, space="PSUM") as psy,
                tc.tile_pool(name="ps_yt", bufs=2, space="PSUM") as psyt,
            ):
                def project_tile(t, xt):
                    """32 matmuls y[b, c, o] + 2 transposes -> yT slices."""
                    psy_t = psy.tile([P, CHUNKS * OUT], F32, tag="psy")
                    for c in range(CHUNKS):
                        for h in range(2):
                            kh = 2 * c + h
                            nc.tensor.matmul(
                                psy_t[:, c * OUT:(c + 1) * OUT],
                                lhsT=xt[:, kh * P:(kh + 1) * P],
                                rhs=w_sb[:, h * CHUNKS * OUT + c * OUT:
                                         h * CHUNKS * OUT + c * OUT + OUT],
                                start=(c == 0 and h == 0),
                                stop=(c == CHUNKS - 1 and h == 1),
                            )
                    y_sb = ysp.tile([P, CHUNKS * OUT], BF16, tag="ysb")
                    nc.scalar.copy(y_sb[:, :], psy_t[:, :])
                    ps_t = psyt.tile([GW, 2 * P], BF16, tag="psyt")
                    nc.tensor.transpose(
                        ps_t[:, 0:P], y_sb[:, 0:GW], id_bf[:, :]
                    )
                    nc.tensor.transpose(
                        ps_t[:, P:2 * P], y_sb[:, GW:2 * GW], id_bf[:, :]
                    )
                    nc.scalar.copy(yT0[:, t * P:(t + 1) * P], ps_t[:, 0:P])
                    nc.scalar.copy(yT1[:, t * P:(t + 1) * P], ps_t[:, P:2 * P])

                last_xt = None
                for t in range(TILES):
                    xb = xbp.tile([P, F], BF16, tag="xb")
                    # SWDGE DMA with f32 -> bf16 cast in the datapath
                    nc.gpsimd.dma_start(xb[:, :], x3[:, t:t + 1, :])

                    # per-chunk sum of |x| for this tile (fused abs+reduce)
                    nc.vector.tensor_reduce(
                        red_all[:, t * CHUNKS:(t + 1) * CHUNKS],
                        xb[:, :].rearrange("p (c f) -> p c f", f=CHUNK),
                        axis=AX.X,
                        op=OP.add,
                        apply_absolute_value=True,
                    )
                    if t == TILES - 2:
                        # partial per-chunk totals over tiles 0..14 --
                        # hidden under the stream; only a [128,16] add
                        # remains after the last tile's stats
                        nc.vector.tensor_reduce(
                            acts_e[:, :],
                            red_all[:, 0:(TILES - 1) * CHUNKS].rearrange(
                                "p (t c) -> p c t", c=CHUNKS
                            ),
                            axis=AX.X,
                            op=OP.add,
                        )

                    # transpose 32 blocks in 4 groups of 8: x[b,f] -> xT[f,b]
                    xt = xtp.tile([P, F], BF16, tag="xt")
                    for g in range(NQ):
                        ps = pstr.tile([P, 8 * P], BF16, tag="ps_tr")
                        for j in range(8):
                            k = 8 * g + j
                            nc.tensor.transpose(
                                ps[:, j * P:(j + 1) * P],
                                xb[:, k * P:(k + 1) * P],
                                id_bf[:, :],
                            )
                        nc.scalar.copy(
                            xt[:, g * 8 * P:(g + 1) * 8 * P], ps[:, :]
                        )
                    if t < TILES - 1:
                        project_tile(t, xt)
                    else:
                        last_xt = xt

                # global per-chunk sums for this core -> [1, 16] -> gather
                acts_p = tailp.tile([P, CHUNKS], F32, tag="acts_p")
                nc.vector.tensor_tensor(
                    acts_p[:, :], acts_e[:, :],
                    red_all[:, (TILES - 1) * CHUNKS:TILES * CHUNKS],
                    op=OP.add,
                )
                ps_a = psm.tile([1, CHUNKS], F32, tag="psmisc")
                nc.tensor.matmul(
                    ps_a[:, :], lhsT=ones_col[:, :], rhs=acts_p[:, :],
                    start=True, stop=True,
                )
                cc_sb = tailp.tile([1, CHUNKS], F32, tag="cc_sb")
                nc.vector.tensor_copy(cc_sb[:, :], ps_a[:, :])
                nc.sync.dma_start(cc_in[:, :], cc_sb[:, :])
                nc.gpsimd.collective_compute(
                    "AllGather",
                    OP.bypass,
                    replica_groups=[list(range(N_CORES))],
                    ins=[cc_in.opt()],
                    outs=[cc_out.opt()],
                )

                # last tile's projection overlaps the collective
                project_tile(TILES - 1, last_xt)

            # ---- rank-based top-4 mask + head weights ----
            Sg = tailp.tile([N_CORES, CHUNKS], F32, tag="Sg")
            nc.sync.dma_start(Sg[:, :], cc_out[:, :])

            # S_row [1, 16] = total per-chunk sums
            ps_sr = psm.tile([1, CHUNKS], F32, tag="psmisc")
            nc.tensor.matmul(
                ps_sr[:, :], lhsT=ones_col[0:N_CORES, :], rhs=Sg[:, :],
                start=True, stop=True,
            )
            S_row = tailp.tile([1, CHUNKS], F32, tag="S_row")
            nc.vector.tensor_copy(S_row[:, :], ps_sr[:, :])
            # S_col [16, 1] via transpose + reduce
            ps_sgt = psm.tile([CHUNKS, N_CORES], F32, tag="ps2")
            nc.tensor.transpose(
                ps_sgt[:, :], Sg[:, :], id_f32[0:N_CORES, 0:N_CORES]
            )
            S_col = tailp.tile([CHUNKS, 1], F32, tag="S_col")
            nc.vector.tensor_reduce(
                S_col[:, :], ps_sgt[:, :], axis=AX.X, op=OP.add
            )
            # S_bcast[i, j] = S_j ; cmp[i, j] = (S_j > S_i); rank_i
            ps_bc = psm.tile([CHUNKS, CHUNKS], F32, tag="psmisc")
            nc.tensor.matmul(
                ps_bc[:, :], lhsT=ones_row[:, 0:CHUNKS], rhs=S_row[:, :],
                start=True, stop=True,
            )
            cmp = tailp.tile([CHUNKS, CHUNKS], F32, tag="cmp")
            nc.vector.tensor_scalar(
                cmp[:, :], ps_bc[:, :], S_col[:, :], None, op0=OP.is_gt
            )
            rank = tailp.tile([CHUNKS, 1], F32, tag="rank")
            nc.vector.tensor_reduce(rank[:, :], cmp[:, :], axis=AX.X, op=OP.add)
            mask_col = tailp.tile([CHUNKS, 1], F32, tag="mask_col")
            nc.vector.tensor_scalar(
                mask_col[:, :], rank[:, :], float(TOPK) - 0.5, None, op0=OP.is_lt
            )
            # cond = (max_c S_c > SUM_THRESH)
            m1 = tailp.tile([1, 1], F32, tag="m1")
            nc.vector.tensor_reduce(m1[:, :], S_row[:, :], axis=AX.X, op=OP.max)
            cond = tailp.tile([1, 1], F32, tag="cond")
            nc.vector.tensor_scalar(
                cond[:, :], m1[:, :], float(SUM_THRESH), None, op0=OP.is_gt
            )

            # mask80 per group, cond80, effective stacked head weights
            ps_m0 = psm.tile([GW, 1], F32, tag="psmisc")
            nc.tensor.matmul(
                ps_m0[:, :], lhsT=E0[:, :], rhs=mask_col[:, :],
                start=True, stop=True,
            )
            m80_0 = tailp.tile([GW, 1], F32, tag="m80_0")
            nc.vector.tensor_copy(m80_0[:, :], ps_m0[:, :])
            ps_m1 = psm.tile([GW, 1], F32, tag="ps2")
            nc.tensor.matmul(
                ps_m1[:, :], lhsT=E1[:, :], rhs=mask_col[:, :],
                start=True, stop=True,
            )
            m80_1 = tailp.tile([GW, 1], F32, tag="m80_1")
            nc.vector.tensor_copy(m80_1[:, :], ps_m1[:, :])
            ps_c80 = psm.tile([GW, 1], F32, tag="psmisc")
            nc.tensor.matmul(
                ps_c80[:, :], lhsT=ones_row[:, 0:GW], rhs=cond[:, :],
                start=True, stop=True,
            )
            c80 = tailp.tile([GW, 1], F32, tag="c80")
            nc.vector.tensor_copy(c80[:, :], ps_c80[:, :])

            weff = tailp.tile([GW, OUT], F32, tag="weff")
            nc.vector.tensor_scalar(
                weff[:, :], dwS[:, :], c80[:, :], None, op0=OP.mult
            )
            nc.vector.tensor_tensor(
                weff[:, :], weff[:, :], wbinS[:, :], op=OP.add
            )
            rhs0 = tailp.tile([GW, OUT], BF16, tag="rhs0")
            nc.vector.tensor_scalar(
                rhs0[:, :], weff[:, :], m80_0[:, :], None, op0=OP.mult
            )
            rhs1 = tailp.tile([GW, OUT], BF16, tag="rhs1")
            nc.vector.tensor_scalar(
                rhs1[:, :], weff[:, :], m80_1[:, :], None, op0=OP.mult
            )

            # bias row [1, 160]: (r0 + cond*r1) tiled over the 16 tiles
            ps_c10 = psm.tile([OUT, 1], F32, tag="ps2")
            nc.tensor.matmul(
                ps_c10[:, :], lhsT=ones_row[:, 0:OUT], rhs=cond[:, :],
                start=True, stop=True,
            )
            bias_col = tailp.tile([OUT, 1], F32, tag="bias_col")
            nc.vector.tensor_scalar(
                bias_col[:, :], r1c[:, :], ps_c10[:, :], None, op0=OP.mult
            )
            nc.vector.tensor_tensor(
                bias_col[:, :], bias_col[:, :], r0c[:, :], op=OP.add
            )
            ps_b160 = psm.tile([1, TILES * OUT], F32, tag="psmisc")
            nc.tensor.matmul(
                ps_b160[:, :], lhsT=bias_col[:, :], rhs=IdT10[:, :],
                start=True, stop=True,
            )
            b160 = tailp.tile([1, TILES * OUT], BF16, tag="b160")
            nc.vector.tensor_copy(b160[:, :], ps_b160[:, :])

            # ---- combine + head + bias: 33 matmuls into one PSUM tile ----
            with tc.tile_pool(name="ps_out", bufs=1, space="PSUM") as pso:
                ps_out = pso.tile([P, TILES * OUT], F32, tag="ps_out")
                for t in range(TILES):
                    nc.tensor.matmul(
                        ps_out[:, t * OUT:(t + 1) * OUT],
                        lhsT=yT0[:, t * P:(t + 1) * P],
                        rhs=rhs0[:, :],
                        start=(t == 0),
                        stop=False,
                    )
                    nc.tensor.matmul(
                        ps_out[:, t * OUT:(t + 1) * OUT],
                        lhsT=yT1[:, t * P:(t + 1) * P],
                        rhs=rhs1[:, :],
                        start=False,
                        stop=False,
                    )
                nc.tensor.matmul(
                    ps_out[:, :], lhsT=ones_row_bf[:, :], rhs=b160[:, :],
                    start=False, stop=True,
                )
                out_sb = tailp.tile([P, TILES * OUT], F32, tag="out_sb")
                nc.scalar.copy(out_sb[:, :], ps_out[:, :])
                # one DMA: per partition 16 consecutive rows = 640B contig
                nc.sync.dma_start(
                    out_d.ap().rearrange("(p j) o -> p j o", j=TILES),
                    out_sb[:, :].rearrange("p (j o) -> p j o", o=OUT),
                )

    nc.compile()
    return nc


def _get_nc():
    if "nc" not in _CACHE:
        _CACHE["nc"] = _build()
    return _CACHE["nc"]


def build_in_maps(x, expert_w, expert_b, quant_w, quant_b):
    import ml_dtypes

    x = np.ascontiguousarray(np.asarray(x, dtype=np.float32))
    expert_w = np.asarray(expert_w, dtype=np.float32)
    expert_b = np.asarray(expert_b, dtype=np.float32)
    quant_w = np.asarray(quant_w, dtype=np.float32)
    quant_b = np.asarray(quant_b, dtype=np.float32)

    # host-side weight prep (tiny tensors only)
    # w_sb[p, h*160 + c*10 + o] = expert_w[c, o, h*128 + p]
    wr = expert_w.reshape(CHUNKS, OUT, 2, P)            # c, o, h, p
    w_sb = np.ascontiguousarray(
        wr.transpose(3, 2, 0, 1).reshape(P, 2 * CHUNKS * OUT)
    ).astype(ml_dtypes.bfloat16)
    qmean = np.float32(np.mean(np.abs(quant_w)))
    wbin = (np.sign(quant_w) * qmean).astype(np.float32)
    dw = (quant_w - wbin).astype(np.float32)
    # stacks: wbinS[c'*10+o, o'] = wbin[o', o]  (i.e. wbin.T tiled 8x)
    wbinS = np.ascontiguousarray(np.tile(wbin.T, (8, 1))).astype(np.float32)
    dwS = np.ascontiguousarray(np.tile(dw.T, (8, 1))).astype(np.float32)
    # mask expansion consts
    E0 = np.zeros((CHUNKS, GW), np.float32)
    E1 = np.zeros((CHUNKS, GW), np.float32)
    for cp in range(8):
        E0[cp, cp * OUT:(cp + 1) * OUT] = 1.0
        E1[cp + 8, cp * OUT:(cp + 1) * OUT] = 1.0
    IdT10 = np.tile(np.eye(OUT, dtype=np.float32), (1, TILES))
    IdT10 = np.ascontiguousarray(IdT10)
    r0 = (expert_b @ wbin.T + quant_b).reshape(OUT, 1).astype(np.float32)
    r1 = (expert_b @ dw.T).reshape(OUT, 1).astype(np.float32)

    id_bf = np.eye(P, dtype=ml_dtypes.bfloat16)
    id_f32 = np.eye(P, dtype=np.float32)
    in_maps = []
    for i in range(N_CORES):
        in_maps.append({
            "x": np.ascontiguousarray(x[i * BC:(i + 1) * BC]),
            "w_sb": w_sb,
            "wbinS": wbinS,
            "dwS": dwS,
            "E0": E0,
            "E1": E1,
            "IdT10": IdT10,
            "r0": r0,
            "r1": r1,
            "id_bf": id_bf,
            "id_f32": id_f32,
        })
    return in_maps


def kernel(x, expert_w, expert_b, quant_w, quant_b):
    in_maps = build_in_maps(x, expert_w, expert_b, quant_w, quant_b)
    nc = _get_nc()
    res = run_bass_kernel_spmd(nc, in_maps, core_ids=list(range(N_CORES)))
    out = np.concatenate(
        [np.asarray(res.results[i]["out"]) for i in range(N_CORES)], axis=0
    )
    return out.astype(np.float32)
